# revision 6
# baseline (speedup 1.0000x reference)
"""PIoU (pixel-wise IoU) pairwise matrix kernel for Trainium2, 8 NeuronCores.

Math: for each pair (predicted box n, target box m) the reference samples a
16x16 grid of the joint AABB and evaluates a soft membership
F = sigmoid(k(w/2-|A|)) * sigmoid(k(h/2-|B|)) per box, where (A, B) are the
pixel offsets rotated into the box frame.  Both A and B are *affine* in the
grid coordinates (ug, uh), so the sigmoid arguments k(s/2 -+ A) for all
256 pixels x 4 fields x {P,Q} are produced by ONE K=24 matmul per 128 pairs
against a constant basis.  sigmoid(min(P,Q)) == min(sigmoid(P), sigmoid(Q))
lets ACT read the matmul PSUM directly with no bias work.

Sharding: N (predicted) axis split 8 ways; each core computes a [512m, 64n]
slab (output transposed on host).

Wall-clock structure (axon-tunneled cores, ~50-70ms RTT): the executable is
AOT-compiled once and cached; per call we upload ~180KB of per-box
quantities (the 24x2048 basis is baked into the NEFF as a Const, the
[128,640] P-box slab is broadcast on device from one row), run, and fetch
the uint8-quantized result (256KB, dequantized on host; fetch costs
~25ms/MB on top of the RTT) in the same round trip as the dispatch.
"""

import numpy as np

N = 512
M = 512
G = 16
NPIX = G * G
K_SLOPE = np.float32(10.0)
EPS = np.float32(1e-6)
NC = 8
NLOC = N // NC  # 64 predicted boxes per core
NCHUNK = 4  # m-chunks of 128

_cache = {}

_QORD = ("x0", "x1", "y0", "y1", "cx", "cy", "ct", "st", "khw", "khh")


def _derived(b):
    # b: [K,5] float32 -> per-box derived quantities (all float32)
    cx, cy, w, h, t = (b[:, i].astype(np.float32) for i in range(5))
    c, s = np.cos(t).astype(np.float32), np.sin(t).astype(np.float32)
    hw = np.float32(0.5) * (w * np.abs(c) + h * np.abs(s))
    hh = np.float32(0.5) * (w * np.abs(s) + h * np.abs(c))
    return dict(
        cx=cx, cy=cy, ct=c, st=s,
        khw=(K_SLOPE * np.float32(0.5)) * w, khh=(K_SLOPE * np.float32(0.5)) * h,
        x0=cx - hw, x1=cx + hw, y0=cy - hh, y1=cy + hh,
    )


def _basis():
    # basis [24, 2*NPIX*4]: P-block cols 0..1023 (fields A1,B1,A2,B2 x 256),
    # Q-block cols 1024..2047.  Field f uses rows 3f..3f+2 (P) / 12+3f.. (Q).
    u = ((np.arange(G, dtype=np.float32) + np.float32(0.5)) / np.float32(G))
    Ug = np.tile(u, G)      # pixel p = h*G+g -> u[g]
    Uh = np.repeat(u, G)    # -> u[h]
    basis = np.zeros((24, 8 * NPIX), dtype=np.float32)
    for f in range(4):
        for blk, r0 in ((0, 0), (1, 12)):
            c0 = blk * 4 * NPIX + f * NPIX
            basis[r0 + 3 * f + 0, c0:c0 + NPIX] = 1.0
            basis[r0 + 3 * f + 1, c0:c0 + NPIX] = Ug
            basis[r0 + 3 * f + 2, c0:c0 + NPIX] = Uh
    return basis


def _host_constants(loc_p, loc_t):
    """Build per-core input arrays (all O(N+M) host work)."""
    T = _derived(loc_t)
    # TQ [128, 4 chunks, 10]: per-target quantities, m = j*128 + partition
    TQ = np.empty((128, NCHUNK, len(_QORD)), dtype=np.float32)
    for qi, q in enumerate(_QORD):
        TQ[:, :, qi] = T[q].reshape(NCHUNK, 128).T

    P = _derived(loc_p)
    PBrows = []
    for c in range(NC):
        sl = slice(c * NLOC, (c + 1) * NLOC)
        pb = np.stack([P[q][sl] for q in _QORD], axis=0)  # [10, 64]
        PBrows.append(pb.reshape(1, 10 * NLOC).copy())
    return TQ.reshape(128, NCHUNK * len(_QORD)), PBrows


def _build_nc():
    from contextlib import ExitStack

    import concourse.bacc as bacc
    import concourse.tile as tile
    from concourse import mybir
    from concourse.masks import make_identity

    dt = mybir.dt
    op = mybir.AluOpType
    AF = mybir.ActivationFunctionType
    K = float(K_SLOPE)

    # Bacc (not raw Bass): its finalize() runs generate_event_semaphores,
    # which legalizes Tile's multi-wait sync_info down to <=1 wait per
    # hardware instruction.
    nc = bacc.Bacc(None, target_bir_lowering=False)
    PBrow_d = nc.declare_dram_parameter("PB", [1, 10 * NLOC], dt.float32, isOutput=False)
    TQ_d = nc.declare_dram_parameter("TQ", [128, NCHUNK * 10], dt.float32, isOutput=False)
    BAS_d = nc.inline_tensor(_basis(), name="BASIS")
    OUT_d = nc.declare_dram_parameter("OUT", [M, NLOC], dt.uint8, isOutput=True)

    with tile.TileContext(nc) as tc, ExitStack() as ctx:
        consts = ctx.enter_context(tc.tile_pool(name="consts", bufs=1))
        coeffp = ctx.enter_context(tc.tile_pool(name="coeffp", bufs=2))
        scratch = ctx.enter_context(tc.tile_pool(name="scratch", bufs=2))
        work = ctx.enter_context(tc.tile_pool(name="work", bufs=2))
        accp = ctx.enter_context(tc.tile_pool(name="accp", bufs=2))
        psum = ctx.enter_context(tc.tile_pool(name="psum", bufs=2, space="PSUM"))

        ident = consts.tile([128, 128], dt.float32)
        make_identity(nc, ident[:])
        PBr = consts.tile([1, 10 * NLOC], dt.float32)
        nc.sync.dma_start(out=PBr[:], in_=PBrow_d[:])
        TQ = consts.tile([128, NCHUNK, 10], dt.float32)
        nc.sync.dma_start(out=TQ[:].rearrange("p a b -> p (a b)"), in_=TQ_d[:])
        BAS = consts.tile([24, 8 * NPIX], dt.float32)
        nc.sync.dma_start(out=BAS[:], in_=BAS_d[:])

        # Broadcast the P-box row across all 128 partitions with a K=1
        # matmul against a ones vector (PSUM banks hold 512 f32, so split).
        ones = consts.tile([1, 128], dt.float32)
        nc.vector.memset(ones[:], 1.0)
        PB = consts.tile([128, 10, NLOC], dt.float32)
        PBflat = PB[:].rearrange("p a b -> p (a b)")
        pbb = psum.tile([128, 4 * NPIX], dt.float32, tag="fields", bufs=3)
        for o0, o1 in ((0, 512), (512, 10 * NLOC)):
            nc.tensor.matmul(pbb[:, o0:o1], ones[:], PBr[:, o0:o1], start=True, stop=True)
        nc.vector.tensor_copy(PBflat[:], pbb[:, 0:10 * NLOC])

        def pb(q):
            return PB[:, _QORD.index(q), :]

        def tq(j, q):
            i = _QORD.index(q)
            return TQ[:, j, i:i + 1]

        for j in range(NCHUNK):
            # ---- coefficient slab C [128 m, 24 rows, 64 n] ----
            C = coeffp.tile([128, 24, NLOC], dt.float32, tag="C")
            S = scratch.tile([128, 16, NLOC], dt.float32, tag="S")
            g = nc.vector

            def s(i):
                return S[:, i, :]

            g.tensor_scalar(s(0), pb("x0"), tq(j, "x0"), None, op.min)   # xmin
            g.tensor_scalar(s(1), pb("x1"), tq(j, "x1"), None, op.max)   # xmax
            g.tensor_scalar(s(2), pb("y0"), tq(j, "y0"), None, op.min)   # ymin
            g.tensor_scalar(s(3), pb("y1"), tq(j, "y1"), None, op.max)   # ymax
            g.tensor_tensor(s(4), s(1), s(0), op.subtract)               # sx
            g.tensor_tensor(s(5), s(3), s(2), op.subtract)               # sy
            g.tensor_tensor(s(6), s(0), pb("cx"), op.subtract)           # dxp
            g.tensor_tensor(s(7), s(2), pb("cy"), op.subtract)           # dyp
            # a0p = dxp*ctp + dyp*stp ; b0p = dyp*ctp - dxp*stp
            g.tensor_tensor(s(8), s(6), pb("ct"), op.mult)
            g.tensor_tensor(s(9), s(7), pb("st"), op.mult)
            g.tensor_tensor(s(9), s(8), s(9), op.add)                    # a0p
            g.tensor_tensor(s(8), s(7), pb("ct"), op.mult)
            g.tensor_tensor(s(10), s(6), pb("st"), op.mult)
            g.tensor_tensor(s(10), s(8), s(10), op.subtract)             # b0p

            def c(r):
                return C[:, r, :]

            # field A1 (const rows): P = khw_p - K*a0p ; Q = khw_p + K*a0p
            g.scalar_tensor_tensor(c(0), s(9), -K, pb("khw"), op.mult, op.add)
            g.scalar_tensor_tensor(c(12), s(9), K, pb("khw"), op.mult, op.add)
            # a1p = sx*ctp -> rows 1/13 ; a2p = sy*stp -> rows 2/14
            g.tensor_tensor(s(8), s(4), pb("ct"), op.mult)
            g.tensor_scalar(c(1), s(8), -K, None, op.mult)
            g.tensor_scalar(c(13), s(8), K, None, op.mult)
            g.tensor_tensor(s(8), s(5), pb("st"), op.mult)
            g.tensor_scalar(c(2), s(8), -K, None, op.mult)
            g.tensor_scalar(c(14), s(8), K, None, op.mult)
            # field B1 (rows 6-8/18-20; field order is A1,A2,B1,B2)
            g.scalar_tensor_tensor(c(6), s(10), -K, pb("khh"), op.mult, op.add)
            g.scalar_tensor_tensor(c(18), s(10), K, pb("khh"), op.mult, op.add)
            # b1p = -sx*stp: s8 = sx*stp -> P row = +K*s8, Q row = -K*s8
            g.tensor_tensor(s(8), s(4), pb("st"), op.mult)
            g.tensor_scalar(c(7), s(8), K, None, op.mult)
            g.tensor_scalar(c(19), s(8), -K, None, op.mult)
            # b2p = sy*ctp
            g.tensor_tensor(s(8), s(5), pb("ct"), op.mult)
            g.tensor_scalar(c(8), s(8), -K, None, op.mult)
            g.tensor_scalar(c(20), s(8), K, None, op.mult)
            # target box: dxt/dyt
            g.tensor_scalar(s(12), s(0), tq(j, "cx"), None, op.subtract)
            g.tensor_scalar(s(13), s(2), tq(j, "cy"), None, op.subtract)
            # a0t = dxt*ctt + dyt*stt
            g.tensor_scalar(s(8), s(12), tq(j, "ct"), None, op.mult)
            g.tensor_scalar(s(14), s(13), tq(j, "st"), None, op.mult)
            g.tensor_tensor(s(14), s(8), s(14), op.add)
            # b0t = dyt*ctt - dxt*stt
            g.tensor_scalar(s(8), s(13), tq(j, "ct"), None, op.mult)
            g.tensor_scalar(s(15), s(12), tq(j, "st"), None, op.mult)
            g.tensor_tensor(s(15), s(8), s(15), op.subtract)
            # field A2 const rows (rows 3-5/15-17)
            g.tensor_scalar(c(3), s(14), -K, tq(j, "khw"), op.mult, op.add)
            g.tensor_scalar(c(15), s(14), K, tq(j, "khw"), op.mult, op.add)
            # a1t = sx*ctt ; a2t = sy*stt
            g.tensor_scalar(s(8), s(4), tq(j, "ct"), None, op.mult)
            g.tensor_scalar(c(4), s(8), -K, None, op.mult)
            g.tensor_scalar(c(16), s(8), K, None, op.mult)
            g.tensor_scalar(s(8), s(5), tq(j, "st"), None, op.mult)
            g.tensor_scalar(c(5), s(8), -K, None, op.mult)
            g.tensor_scalar(c(17), s(8), K, None, op.mult)
            # field B2 const rows
            g.tensor_scalar(c(9), s(15), -K, tq(j, "khh"), op.mult, op.add)
            g.tensor_scalar(c(21), s(15), K, tq(j, "khh"), op.mult, op.add)
            # b1t = -sx*stt ; b2t = sy*ctt
            g.tensor_scalar(s(8), s(4), tq(j, "st"), None, op.mult)
            g.tensor_scalar(c(10), s(8), K, None, op.mult)
            g.tensor_scalar(c(22), s(8), -K, None, op.mult)
            g.tensor_scalar(s(8), s(5), tq(j, "ct"), None, op.mult)
            g.tensor_scalar(c(11), s(8), -K, None, op.mult)
            g.tensor_scalar(c(23), s(8), K, None, op.mult)

            Ssum = accp.tile([128, NLOC], dt.float32, tag="Ssum")
            Isum = accp.tile([128, NLOC], dt.float32, tag="Isum")

            # ---- main loop over the 64 predicted boxes of this core ----
            for n in range(NLOC):
                coeffT = psum.tile([24, 128], dt.float32, tag="coeffT")
                nc.tensor.transpose(coeffT[:], C[:, :, n], ident[:])
                lhsT = work.tile([24, 128], dt.float32, tag="lhsT")
                nc.vector.tensor_copy(lhsT[:], coeffT[:])

                fieldP = psum.tile([128, 4 * NPIX], dt.float32, tag="fields", bufs=3)
                fieldQ = psum.tile([128, 4 * NPIX], dt.float32, tag="fields", bufs=3)
                for q in range(2):
                    nc.tensor.matmul(
                        fieldP[:, q * 512:(q + 1) * 512],
                        lhsT[:], BAS[:, q * 512:(q + 1) * 512],
                        start=True, stop=True)
                for q in range(2):
                    nc.tensor.matmul(
                        fieldQ[:, q * 512:(q + 1) * 512],
                        lhsT[:], BAS[:, 1024 + q * 512:1024 + (q + 1) * 512],
                        start=True, stop=True)
                sigP = work.tile([128, 4 * NPIX], dt.bfloat16, tag="sigP")
                nc.scalar.activation(sigP[:], fieldP[:], AF.Sigmoid)
                sigQ = work.tile([128, 4 * NPIX], dt.bfloat16, tag="sigQ")
                nc.scalar.activation(sigQ[:], fieldQ[:], AF.Sigmoid)

                vmin = work.tile([128, 4, NPIX], dt.bfloat16, tag="vmin")
                nc.vector.tensor_tensor(
                    vmin[:].rearrange("p f q -> p (f q)"),
                    sigP[:], sigQ[:], op.min)

                vflat = vmin[:].rearrange("p f q -> p (f q)")
                Fp = work.tile([128, 2 * NPIX], dt.bfloat16, tag="Fp")
                nc.vector.tensor_mul(Fp[:], vflat[:, 0:2 * NPIX], vflat[:, 2 * NPIX:4 * NPIX])
                nc.vector.tensor_reduce(
                    Ssum[:, n:n + 1], Fp[:], mybir.AxisListType.X, op.add)
                F12 = work.tile([128, NPIX], dt.bfloat16, tag="F12")
                nc.vector.tensor_mul(F12[:], Fp[:, 0:NPIX], Fp[:, NPIX:2 * NPIX])
                nc.vector.tensor_reduce(
                    Isum[:, n:n + 1], F12[:], mybir.AxisListType.X, op.add)

            # ---- epilogue: piou = inter / (stot - inter + eps) ----
            union = scratch.tile([128, NLOC], dt.float32, tag="union")
            nc.vector.scalar_tensor_tensor(
                union[:], Isum[:], -1.0, Ssum[:], op.mult, op.add)
            nc.vector.tensor_scalar(union[:], union[:], float(EPS), None, op.add)
            rec = scratch.tile([128, NLOC], dt.float32, tag="rec")
            nc.vector.reciprocal(rec[:], union[:])
            pf = scratch.tile([128, NLOC], dt.float32, tag="pf")
            nc.vector.tensor_tensor(pf[:], Isum[:], rec[:], op.mult)
            # quantize to uint8: piou*255 saturated to [0,255]; the f32->u8
            # cast on the DVE write rounds to nearest; host divides by 255
            piou = accp.tile([128, NLOC], dt.uint8, tag="piou")
            nc.vector.tensor_scalar(piou[:], pf[:], 255.0, 255.0, op.mult, op.min)
            nc.sync.dma_start(out=OUT_d[j * 128:(j + 1) * 128, :], in_=piou[:])

    nc.finalize()
    return nc


def _get_compiled():
    if "nc" not in _cache:
        _cache["nc"] = _build_nc()
    return _cache["nc"]


def _get_runner():
    """AOT-compile the 8-core executable once; per call only upload inputs,
    execute, and fetch.  run_bass_kernel_spmd rebuilds its jit closure every
    call (re-running BIR verify + walrus, ~0.5s), so we pin the compiled
    executable here and dispatch it directly."""
    if "runner" in _cache:
        return _cache["runner"]

    import jax
    from jax.sharding import Mesh, PartitionSpec
    try:
        from jax.experimental.shard_map import shard_map
    except ImportError:  # newer jax
        from jax import shard_map
    from concourse import mybir
    from concourse.bass2jax import (
        _bass_exec_p,
        fast_dispatch_compile,
        install_neuronx_cc_hook,
        partition_id_tensor,
    )

    install_neuronx_cc_hook()
    nc = _get_compiled()

    partition_name = nc.partition_id_tensor.name if nc.partition_id_tensor else None
    in_names, out_names, out_avals = [], [], []
    for alloc in nc.m.functions[0].allocations:
        if not isinstance(alloc, mybir.MemoryLocationSet):
            continue
        name = alloc.memorylocations[0].name
        if alloc.kind == "ExternalInput":
            if name != partition_name:
                in_names.append(name)
        elif alloc.kind == "ExternalOutput":
            out_names.append(name)
            out_avals.append(
                jax.core.ShapedArray(tuple(alloc.tensor_shape), mybir.dt.np(alloc.dtype))
            )
    all_names = in_names + ([partition_name] if partition_name else [])
    shapes = {
        alloc.memorylocations[0].name: (tuple(alloc.tensor_shape), mybir.dt.np(alloc.dtype))
        for alloc in nc.m.functions[0].allocations
        if isinstance(alloc, mybir.MemoryLocationSet) and alloc.kind == "ExternalInput"
    }

    # The kernel writes every OUT element, so no zero-initialized output
    # operands are needed (run_bass_via_pjrt donates them only to give
    # unwritten elements a deterministic value).
    def _body(*args):
        operands = list(args)
        if partition_name is not None:
            operands.append(partition_id_tensor())
        outs = _bass_exec_p.bind(
            *operands,
            out_avals=tuple(out_avals),
            in_names=tuple(all_names),
            out_names=tuple(out_names),
            lowering_input_output_aliases=(),
            sim_require_finite=True,
            sim_require_nnan=True,
            nc=nc,
        )
        return tuple(outs)

    devices = jax.devices()[:NC]
    mesh = Mesh(np.asarray(devices), ("core",))
    smapped = shard_map(
        _body,
        mesh=mesh,
        in_specs=(PartitionSpec("core"),) * len(in_names),
        out_specs=(PartitionSpec("core"),) * len(out_names),
        check_rep=False,
    )
    arg_specs = [
        jax.ShapeDtypeStruct((NC * shapes[nm][0][0],) + shapes[nm][0][1:], shapes[nm][1])
        for nm in in_names
    ]
    compiled = fast_dispatch_compile(lambda: jax.jit(smapped).lower(*arg_specs).compile())

    def runner(in_map):
        # in_map: name -> concatenated-along-axis-0 global array
        outs = compiled(*[in_map[nm] for nm in in_names])
        return np.asarray(outs[0])

    _cache["runner"] = runner
    return runner


def kernel(loc_p, loc_t, grid):
    assert int(grid) == G
    loc_p = np.asarray(loc_p, dtype=np.float32)
    loc_t = np.asarray(loc_t, dtype=np.float32)
    TQ, PBrows = _host_constants(loc_p, loc_t)

    r = None
    try:
        r = _get_runner()({
            "PB": np.concatenate(PBrows, axis=0),
            "TQ": np.concatenate([TQ] * NC, axis=0),
        })
    except Exception:
        r = None

    if r is None:  # fallback: stock spmd path (same NEFF, slower dispatch)
        from concourse.bass_utils import run_bass_kernel_spmd

        nc = _get_compiled()
        in_maps = [{"PB": PBrows[c], "TQ": TQ} for c in range(NC)]
        res = run_bass_kernel_spmd(nc, in_maps, core_ids=list(range(NC)))
        r = np.concatenate([res.results[c]["OUT"] for c in range(NC)], axis=0)

    # [NC, M, NLOC] -> [NC, NLOC, M] = [N, M]; dequantize uint8 -> f32
    r = np.ascontiguousarray(
        np.asarray(r).reshape(NC, M, NLOC).transpose(0, 2, 1), dtype=np.float32
    )
    r *= np.float32(1.0 / 255.0)
    return r.reshape(N, M)
